# revision 6
# baseline (speedup 1.0000x reference)
"""Max-dilated conv2d kernel for Trainium2 (Bass/Tile), 8-core data parallel.

out[b,oc,oh,ow] = max_{ic,kh,kw} x[b,ic,oh+2*kh, ow+2*kw] * w[oc,ic,kh,kw]
x (8,32,68,68) f32, w (32,32,3,3) f32, out (8,32,64,64) f32; stride 1, dil 2.

Strategy (vs the 361us fp32 fused-STT baseline):
  - everything in fp16 (rel err ~6e-4, gate is 2e-2): VectorE tensor_scalar
    mult runs in 4x DVE mode (~1.3us/plane), tensor_tensor max in 2x mode
    (~2.2us/plane 128x4096), vs 4.5us/plane for the fp32 fused STT.
  - ScalarE produces ~2/3 of the product planes (~3.8us/plane) while
    VectorE does every max-accumulate; plane maxes are pair-fused into
    [128,2,64,64] tensor_tensor ops against a 2-slot accumulator.
  - PE/Pool cannot help: matmul only sums into fp32 PSUM, and the Pool
    engine rejects TensorTensor/TensorScalar at ISA level on TRN2.
  - partition layout p = icq*32 + oc (4 ic x 32 oc), 8 ic-groups of 4;
    x replicated via fp16 broadcast DMA from DRAM; icq folded 128->64->32
    at the end, quarter-row interleaved with the last planes.

Like v2 (fp16 TS/Act mults + fp16 TT max) but:
  - acc has 3 slots [128, 3, OH, OW]; groups of 9 planes fold as exactly
    3 triple-fused tensor_tensor maxes (free size 12288 amortizes per-op
    overhead and sem waits; each triple = 2 ScalarE + 1 VectorE product).
  - group 0 stays per-plane (row-split) for fast startup.
  - final merge of the 2 acc slots, then the 128->64->32 partition fold.
"""

import sys

sys.path.insert(0, "/opt/trn_rl_repo")

import numpy as np

import concourse.bacc as bacc
import concourse.tile as tile
from concourse import mybir
from concourse import bass_utils

IC, OC, K = 32, 32, 3
H = W = 68
OH = OW = 64
DH = DW = 2
NCORES = 8
NGROUPS = 8
PLANES = NGROUPS * K * K  # 72
F32 = mybir.dt.float32
F16 = mybir.dt.float16

# per-group count of planes whose mult runs on ScalarE (rest on VectorE TS)
ACT_PER_GROUP = [4, 6, 6, 6, 6, 6, 6, 6]  # sum 46

_cache: dict = {}


def _build():
    if "final" in _cache:
        return _cache["final"]

    nc = bacc.Bacc("TRN2", debug=False, num_devices=NCORES)
    x_d = nc.dram_tensor("x", [IC, H, W], F16, kind="ExternalInput").ap()
    wv_d = nc.dram_tensor("wv", [128, PLANES], F32, kind="ExternalInput").ap()
    out_d = nc.dram_tensor("out", [OC, OH, OW], F32, kind="ExternalOutput").ap()

    with tile.TileContext(nc) as tc:
        with (
            tc.tile_pool(name="const", bufs=1) as cpool,
            tc.tile_pool(name="xrep", bufs=3) as xpool,
            tc.tile_pool(name="work", bufs=1) as wpool,
        ):
            wv_sb = cpool.tile([128, PLANES], F32, tag="wv")
            nc.sync.dma_start(wv_sb[:, :], wv_d[:, :])

            # two accumulator slots; merged at the end
            acc = wpool.tile([128, 3, OH, OW], F16, tag="acc")

            dma_engines = [nc.sync, nc.gpsimd]
            ei = 0

            def bcast_group(h, fine):
                xr = xpool.tile([128, H, W], F16, tag="xr")
                nonlocal ei
                if fine:
                    splits = [(s * 17, (s + 1) * 17) for s in range(4)]
                    engs = [nc.sync, nc.gpsimd, nc.scalar]
                else:
                    splits = [(0, 34), (34, 68)]
                    engs = dma_engines
                for r0, r1 in splits:
                    for icq in range(4):
                        src = x_d[h * 4 + icq].unsqueeze(0).broadcast_to([32, H, W])
                        engs[ei % len(engs)].dma_start(
                            xr[icq * 32 : (icq + 1) * 32, r0:r1], src[:, r0:r1]
                        )
                        ei += 1
                return xr

            def mul_into(dst, xr, kh, kw, wcol, engine, a=0, b=OH):
                view = xr[:, DH * kh + a : DH * kh + b, DW * kw : DW * kw + OW]
                if engine == "act":
                    nc.scalar.mul(dst, view, wcol)
                else:
                    nc.vector.tensor_scalar_mul(dst, view, wcol)

            # ---------------- group 0: per-plane, row-split startup ----------
            xr = bcast_group(0, fine=True)
            for k in range(K * K):
                kh, kw = divmod(k, K)
                wcol = wv_sb[:, k : k + 1]
                slot = k % 3
                on_act = 3 <= k < 3 + ACT_PER_GROUP[0]
                if k < 4:
                    ksplits = [(0, 13), (13, 30), (30, 47), (47, 64)]
                elif k < 6:
                    ksplits = [(0, 30), (30, 64)]
                else:
                    ksplits = [(0, 64)]
                for a, b in ksplits:
                    accw = acc[:, slot, a:b, :]
                    if k < 3:
                        # first plane of each slot: write acc directly (DVE TS)
                        mul_into(accw, xr, kh, kw, wcol, "vec", a, b)
                    else:
                        prod = xpool.tile(
                            [128, b - a, OW], F16, tag="prod0", name="prod0", bufs=3
                        )
                        mul_into(prod[:], xr, kh, kw, wcol, "act" if on_act else "vec", a, b)
                        nc.vector.tensor_max(accw, accw, prod[:])

            # ---------------- groups 1..7: triple-fused ------------------
            # 9 planes = 3 triples of (act, act, vec): one TT per triple,
            # perfectly uniform per-group engine balance.
            for h in range(1, NGROUPS):
                xr = bcast_group(h, fine=False)
                last = h == NGROUPS - 1
                triples = [(0, 1, 6), (2, 3, 7), (4, 5, 8)]
                engines = ["act"] * 6 + ["vec"] * 3
                n_t = 2 if last else 3
                for t in range(n_t):
                    pp = xpool.tile(
                        [128, 3, OH, OW], F16, tag="pp", name="pp", bufs=4
                    )
                    for si, k in enumerate(triples[t]):
                        kh, kw = divmod(k, K)
                        j = h * (K * K) + k
                        mul_into(
                            pp[:, si, :, :], xr, kh, kw,
                            wv_sb[:, j : j + 1], engines[k],
                        )
                    nc.vector.tensor_max(acc[:, :, :, :], acc[:, :, :, :], pp[:, :, :, :])

                if last:
                    # final triple quarter-split, interleaved with the
                    # phase-batched slot-merge + partition fold
                    pp = xpool.tile([128, 3, OH, OW], F16, tag="pp", name="pp", bufs=4)
                    for si, k in enumerate(triples[2]):
                        kh, kw = divmod(k, K)
                        j = h * (K * K) + k
                        mul_into(pp[:, si, :, :], xr, kh, kw, wv_sb[:, j : j + 1], engines[k])

                    t64 = wpool.tile([64, OH, OW], F16, tag="t64")
                    out_sb = wpool.tile([32, OH, OW], F16, tag="out_sb")
                    quarters = [(0, 16), (16, 32), (32, 48), (48, 64)]
                    for hi, (a, b) in enumerate(quarters):
                        nc.vector.tensor_max(
                            acc[:, :, a:b, :], acc[:, :, a:b, :], pp[:, :, a:b, :]
                        )
                        nc.vector.tensor_max(
                            acc[:, 1, a:b, :], acc[:, 1, a:b, :], acc[:, 2, a:b, :]
                        )
                        nc.vector.tensor_max(
                            acc[:, 0, a:b, :], acc[:, 0, a:b, :], acc[:, 1, a:b, :]
                        )
                        (nc.sync if hi % 2 == 0 else nc.scalar).dma_start(
                            t64[:, a:b, :], acc[64:128, 0, a:b, :]
                        )
                    t32s = []
                    for hi, (a, b) in enumerate(quarters):
                        nc.vector.tensor_max(
                            t64[:, a:b, :], t64[:, a:b, :], acc[0:64, 0, a:b, :]
                        )
                        t32 = wpool.tile([32, 16, OW], F16, tag=f"t32_{hi}")
                        (nc.scalar if hi % 2 == 0 else nc.sync).dma_start(
                            t32[:, :, :], t64[32:64, a:b, :]
                        )
                        t32s.append(t32)
                    for hi, (a, b) in enumerate(quarters):
                        nc.vector.tensor_max(
                            out_sb[:, a:b, :], t32s[hi][:, :, :], t64[0:32, a:b, :]
                        )
                        nc.gpsimd.dma_start(out_d[:, a:b, :], out_sb[:, a:b, :])

    nc.compile()
    _cache["final"] = nc
    return nc


def _make_wv(w: np.ndarray) -> np.ndarray:
    wr = w.reshape(OC, NGROUPS, 4, K * K)
    wv = wr.transpose(2, 0, 1, 3).reshape(4 * OC, PLANES)
    return np.ascontiguousarray(wv, dtype=np.float32)


def _ensure_axon_hooks_module():
    try:
        import antenv.axon_hooks  # noqa: F401
    except Exception:
        import types

        mod = types.ModuleType("antenv.axon_hooks")
        mod._hook = None
        mod.get_axon_ntff_profile_hook = lambda: getattr(mod, "_hook", None)
        mod.set_axon_ntff_profile_hook = lambda h: setattr(mod, "_hook", h)
        sys.modules["antenv.axon_hooks"] = mod
        try:
            import antenv

            antenv.axon_hooks = mod
        except Exception:
            pass


def kernel(x, weight, stride_h=1, stride_w=1, dilation_h=2, dilation_w=2):
    _ensure_axon_hooks_module()
    x = np.asarray(x, dtype=np.float32)
    w = np.ascontiguousarray(np.asarray(weight, dtype=np.float32))
    assert int(stride_h) == 1 and int(stride_w) == 1
    assert int(dilation_h) == DH and int(dilation_w) == DW
    B = x.shape[0]
    assert x.shape == (B, IC, H, W) and w.shape == (OC, IC, K, K)
    assert B == NCORES

    x16 = np.ascontiguousarray(x.astype(np.float16))
    wv = _make_wv(w)
    nc = _build()
    in_maps = [{"x": x16[b], "wv": wv} for b in range(B)]
    res = bass_utils.run_bass_kernel_spmd(nc, in_maps, core_ids=list(range(B)))
    out = np.stack([r["out"] for r in res.results], axis=0)
    return out.astype(np.float32)


def run_traced(x, weight, **trace_kwargs):
    _ensure_axon_hooks_module()
    x = np.asarray(x, dtype=np.float32)
    w = np.ascontiguousarray(np.asarray(weight, dtype=np.float32))
    x16 = np.ascontiguousarray(x.astype(np.float16))
    wv = _make_wv(w)
    nc = _build()
    in_maps = [{"x": x16[b], "wv": wv} for b in range(x.shape[0])]
    res = bass_utils.run_bass_kernel_spmd(
        nc, in_maps, core_ids=list(range(x.shape[0])), trace=True, **trace_kwargs
    )
    out = np.stack([r["out"] for r in res.results], axis=0)
    return out.astype(np.float32), res


# revision 7
# speedup vs baseline: 1.1937x; 1.1937x over previous
"""Max-dilated conv2d kernel for Trainium2 (Bass/Tile), 8-core data parallel.

out[b,oc,oh,ow] = max_{ic,kh,kw} x[b,ic,oh+2*kh, ow+2*kw] * w[oc,ic,kh,kw]
x (8,32,68,68) f32, w (32,32,3,3) f32, out (8,32,64,64) f32; stride 1, dil 2.

Strategy (vs the 361us fp32 fused-STT baseline):
  - everything in fp16 (rel err ~6e-4, gate is 2e-2): VectorE tensor_scalar
    mult runs in 4x DVE mode (~1.3us/plane), tensor_tensor max in 2x mode
    (~2.2us/plane 128x4096), vs 4.5us/plane for the fp32 fused STT.
  - ScalarE produces ~2/3 of the product planes (~3.8us/plane) while
    VectorE does every max-accumulate; plane maxes are pair-fused into
    [128,2,64,64] tensor_tensor ops against a 2-slot accumulator.
  - PE/Pool cannot help: matmul only sums into fp32 PSUM, and the Pool
    engine rejects TensorTensor/TensorScalar at ISA level on TRN2.
  - partition layout p = icq*32 + oc (4 ic x 32 oc), 8 ic-groups of 4;
    x replicated via fp16 broadcast DMA from DRAM; icq folded 128->64->32
    at the end, quarter-row interleaved with the last planes.

Like v2 (fp16 TS/Act mults + fp16 TT max) but:
  - acc has 3 slots [128, 3, OH, OW]; groups of 9 planes fold as exactly
    3 triple-fused tensor_tensor maxes (free size 12288 amortizes per-op
    overhead and sem waits; each triple = 2 ScalarE + 1 VectorE product).
  - group 0 stays per-plane (row-split) for fast startup.
  - final merge of the 2 acc slots, then the 128->64->32 partition fold.
"""

import sys

sys.path.insert(0, "/opt/trn_rl_repo")

import numpy as np

import concourse.bacc as bacc
import concourse.tile as tile
from concourse import mybir
from concourse import bass_utils

IC, OC, K = 32, 32, 3
H = W = 68
OH = OW = 64
DH = DW = 2
NCORES = 8
NGROUPS = 8
PLANES = NGROUPS * K * K  # 72
F32 = mybir.dt.float32
F16 = mybir.dt.float16

# per-group count of planes whose mult runs on ScalarE (rest on VectorE TS)
ACT_PER_GROUP = [4, 6, 6, 6, 6, 6, 6, 6]  # sum 46

_cache: dict = {}


def _build():
    if "final" in _cache:
        return _cache["final"]

    nc = bacc.Bacc("TRN2", debug=False, num_devices=NCORES)
    x_d = nc.dram_tensor("x", [IC, H, W], F16, kind="ExternalInput").ap()
    wv_d = nc.dram_tensor("wv", [128, PLANES], F32, kind="ExternalInput").ap()
    out_d = nc.dram_tensor("out", [OC, OH, OW], F32, kind="ExternalOutput").ap()

    with tile.TileContext(nc) as tc:
        with (
            tc.tile_pool(name="const", bufs=1) as cpool,
            tc.tile_pool(name="xrep", bufs=3) as xpool,
            tc.tile_pool(name="work", bufs=1) as wpool,
        ):
            wv_sb = cpool.tile([128, PLANES], F32, tag="wv")
            nc.sync.dma_start(wv_sb[:, :], wv_d[:, :])

            # two accumulator slots; merged at the end
            acc = wpool.tile([128, 3, OH, OW], F16, tag="acc")

            dma_engines = [nc.sync, nc.gpsimd]
            ei = 0

            def bcast_group(h, fine):
                xr = xpool.tile([128, H, W], F16, tag="xr")
                nonlocal ei
                if fine:
                    splits = [(s * 17, (s + 1) * 17) for s in range(4)]
                    engs = [nc.sync, nc.gpsimd, nc.scalar]
                else:
                    splits = [(0, 34), (34, 68)]
                    engs = dma_engines
                for r0, r1 in splits:
                    for icq in range(4):
                        src = x_d[h * 4 + icq].unsqueeze(0).broadcast_to([32, H, W])
                        engs[ei % len(engs)].dma_start(
                            xr[icq * 32 : (icq + 1) * 32, r0:r1], src[:, r0:r1]
                        )
                        ei += 1
                return xr

            def mul_into(dst, xr, kh, kw, wcol, engine, a=0, b=OH):
                view = xr[:, DH * kh + a : DH * kh + b, DW * kw : DW * kw + OW]
                if engine == "act":
                    nc.scalar.mul(dst, view, wcol)
                else:
                    nc.vector.tensor_scalar_mul(dst, view, wcol)

            # ---------------- group 0: per-plane, row-split startup ----------
            xr = bcast_group(0, fine=True)
            for k in range(K * K):
                kh, kw = divmod(k, K)
                wcol = wv_sb[:, k : k + 1]
                slot = k % 3
                on_act = 3 <= k < 3 + ACT_PER_GROUP[0]
                if k < 4:
                    ksplits = [(0, 13), (13, 30), (30, 47), (47, 64)]
                elif k < 6:
                    ksplits = [(0, 30), (30, 64)]
                else:
                    ksplits = [(0, 64)]
                for a, b in ksplits:
                    accw = acc[:, slot, a:b, :]
                    if k < 3:
                        # first plane of each slot: write acc directly (DVE TS)
                        mul_into(accw, xr, kh, kw, wcol, "vec", a, b)
                    else:
                        prod = xpool.tile(
                            [128, b - a, OW], F16, tag="prod0", name="prod0", bufs=3
                        )
                        mul_into(prod[:], xr, kh, kw, wcol, "act" if on_act else "vec", a, b)
                        nc.vector.tensor_max(accw, accw, prod[:])

            # ---------------- groups 1..7: triple-fused ------------------
            # 9 planes = 3 triples of (act, act, vec): one TT per triple,
            # perfectly uniform per-group engine balance.
            for h in range(1, NGROUPS):
                xr = bcast_group(h, fine=False)
                last = h == NGROUPS - 1
                triples = [(0, 1, 6), (2, 3, 7), (4, 5, 8)]
                engines = ["act"] * 6 + ["vec"] * 3
                n_t = 2 if last else 3
                for t in range(n_t):
                    pp = xpool.tile(
                        [128, 3, OH, OW], F16, tag="pp", name="pp", bufs=4
                    )
                    for si, k in enumerate(triples[t]):
                        kh, kw = divmod(k, K)
                        j = h * (K * K) + k
                        mul_into(
                            pp[:, si, :, :], xr, kh, kw,
                            wv_sb[:, j : j + 1], engines[k],
                        )
                    nc.vector.tensor_max(acc[:, :, :, :], acc[:, :, :, :], pp[:, :, :, :])

                if last:
                    # final triple quarter-split, interleaved with the
                    # phase-batched slot-merge + partition fold
                    pp = xpool.tile([128, 3, OH, OW], F16, tag="pp", name="pp", bufs=4)
                    for si, k in enumerate(triples[2]):
                        kh, kw = divmod(k, K)
                        j = h * (K * K) + k
                        mul_into(pp[:, si, :, :], xr, kh, kw, wv_sb[:, j : j + 1], engines[k])

                    t64 = wpool.tile([64, OH, OW], F16, tag="t64")
                    out_sb = wpool.tile([32, OH, OW], F16, tag="out_sb")
                    quarters = [(0, 16), (16, 32), (32, 48), (48, 64)]
                    for hi, (a, b) in enumerate(quarters):
                        nc.vector.tensor_max(
                            acc[:, :, a:b, :], acc[:, :, a:b, :], pp[:, :, a:b, :]
                        )
                        nc.vector.tensor_max(
                            acc[:, 1, a:b, :], acc[:, 1, a:b, :], acc[:, 2, a:b, :]
                        )
                        nc.vector.tensor_max(
                            acc[:, 0, a:b, :], acc[:, 0, a:b, :], acc[:, 1, a:b, :]
                        )
                        (nc.sync if hi % 2 == 0 else nc.scalar).dma_start(
                            t64[:, a:b, :], acc[64:128, 0, a:b, :]
                        )
                    t32s = []
                    for hi, (a, b) in enumerate(quarters):
                        nc.vector.tensor_max(
                            t64[:, a:b, :], t64[:, a:b, :], acc[0:64, 0, a:b, :]
                        )
                        t32 = wpool.tile([32, 16, OW], F16, tag=f"t32_{hi}")
                        (nc.scalar if hi % 2 == 0 else nc.sync).dma_start(
                            t32[:, :, :], t64[32:64, a:b, :]
                        )
                        t32s.append(t32)
                    out32 = wpool.tile([32, 16, OW], F32, tag="out32")
                    for hi, (a, b) in enumerate(quarters):
                        if hi < 3:
                            nc.vector.tensor_max(
                                out_sb[:, a:b, :], t32s[hi][:, :, :], t64[0:32, a:b, :]
                            )
                            nc.gpsimd.dma_start(out_d[:, a:b, :], out_sb[:, a:b, :])
                        else:
                            # critical last quarter: fp32 fold + fast HWDGE out
                            # (skips the ~1us SWDGE prep on the finish path)
                            nc.vector.tensor_max(
                                out32[:, :, :], t32s[hi][:, :, :], t64[0:32, a:b, :]
                            )
                            nc.sync.dma_start(out_d[:, a:b, :], out32[:, :, :])

    nc.compile()
    _cache["final"] = nc
    return nc


def _make_wv(w: np.ndarray) -> np.ndarray:
    wr = w.reshape(OC, NGROUPS, 4, K * K)
    wv = wr.transpose(2, 0, 1, 3).reshape(4 * OC, PLANES)
    return np.ascontiguousarray(wv, dtype=np.float32)


def _ensure_axon_hooks_module():
    try:
        import antenv.axon_hooks  # noqa: F401
    except Exception:
        import types

        mod = types.ModuleType("antenv.axon_hooks")
        mod._hook = None
        mod.get_axon_ntff_profile_hook = lambda: getattr(mod, "_hook", None)
        mod.set_axon_ntff_profile_hook = lambda h: setattr(mod, "_hook", h)
        sys.modules["antenv.axon_hooks"] = mod
        try:
            import antenv

            antenv.axon_hooks = mod
        except Exception:
            pass


def kernel(x, weight, stride_h=1, stride_w=1, dilation_h=2, dilation_w=2):
    _ensure_axon_hooks_module()
    x = np.asarray(x, dtype=np.float32)
    w = np.ascontiguousarray(np.asarray(weight, dtype=np.float32))
    assert int(stride_h) == 1 and int(stride_w) == 1
    assert int(dilation_h) == DH and int(dilation_w) == DW
    B = x.shape[0]
    assert x.shape == (B, IC, H, W) and w.shape == (OC, IC, K, K)
    assert B == NCORES

    x16 = np.ascontiguousarray(x.astype(np.float16))
    wv = _make_wv(w)
    nc = _build()
    in_maps = [{"x": x16[b], "wv": wv} for b in range(B)]
    res = bass_utils.run_bass_kernel_spmd(nc, in_maps, core_ids=list(range(B)))
    out = np.stack([r["out"] for r in res.results], axis=0)
    return out.astype(np.float32)


def run_traced(x, weight, **trace_kwargs):
    _ensure_axon_hooks_module()
    x = np.asarray(x, dtype=np.float32)
    w = np.ascontiguousarray(np.asarray(weight, dtype=np.float32))
    x16 = np.ascontiguousarray(x.astype(np.float16))
    wv = _make_wv(w)
    nc = _build()
    in_maps = [{"x": x16[b], "wv": wv} for b in range(x.shape[0])]
    res = bass_utils.run_bass_kernel_spmd(
        nc, in_maps, core_ids=list(range(x.shape[0])), trace=True, **trace_kwargs
    )
    out = np.stack([r["out"] for r in res.results], axis=0)
    return out.astype(np.float32), res
